# revision 8
# baseline (speedup 1.0000x reference)
"""AttentionHead kernel for 8 Trainium2 NeuronCores.

Problem (per sample, B=4): x:[256,64,64] -> q/k/v 1x1-conv projections
(+positional encoding on q,k), S = q^T k / 8, softmax over the QUERY axis,
out = attn @ v, then 1x1-conv MLP with Mish + residual.

Sharding: 2 cores per sample, split over the query axis i (2048 queries each).
Softmax normalizes over i, so the per-key denominator den[j] = sum_i exp(S[i,j])
needs one 16 KiB AllReduce per core pair; den then folds into v (v/den) and
everything else is local, with disjoint output halves.

Layout trick: compute S transposed, S[j,i] = (k^T q)[j,i], keys j on partitions.
exp runs PSUM->SBUF with a free per-partition accum (the denominator), and
exp(S)[j,i] is then directly the correct operand layout for both
out[c,i] = sum_j v[c,j]*attnT[j,i] and the MLP — zero on-device transposes.
All matmul operands bf16 (fp32 PSUM accumulation): measured 3.9e-5 rel err.
"""

import numpy as np
import ml_dtypes

import concourse.bass as bass
import concourse.bacc as bacc
import concourse.mybir as mybir
import concourse.tile as tile

BF16 = mybir.dt.bfloat16
F32 = mybir.dt.float32
AF = mybir.ActivationFunctionType
bf16 = ml_dtypes.bfloat16

B, C, H, W = 4, 256, 64, 64
N = H * W            # 4096 pixels
QK = 64
IS = N // 2          # 2048 queries per core
NJT = N // 128       # 32 key tiles
NIB = IS // 512      # 4 i-blocks
N_CORES = 8
REPLICA_GROUPS = [[0, 1], [2, 3], [4, 5], [6, 7]]


def build_program(n_cores: int = N_CORES, enable_asserts: bool = False,
                  use_mish_lut: bool = False) -> bass.Bass:
    nc = bacc.Bacc(
        "TRN2",
        target_bir_lowering=False,
        debug=False,
        enable_asserts=enable_asserts,
        num_devices=n_cores,
    )

    # Per-core inputs (data differs by core; program is identical).
    xb_d = nc.dram_tensor("xb", [C, N], BF16, kind="ExternalInput").ap()
    xq_d = nc.dram_tensor("xq", [C, IS], BF16, kind="ExternalInput").ap()
    xf_d = nc.dram_tensor("xf", [C, IS], F32, kind="ExternalInput").ap()
    pe2q_d = nc.dram_tensor("pe2q", [128, IS], BF16, kind="ExternalInput").ap()
    # Shared weights (same on all cores).
    pe2_d = nc.dram_tensor("pe2", [128, N], BF16, kind="ExternalInput").ap()
    wq2t_d = nc.dram_tensor("wq2t", [128, 256], BF16, kind="ExternalInput").ap()
    wk2t_d = nc.dram_tensor("wk2t", [128, 256], BF16, kind="ExternalInput").ap()
    wvt_d = nc.dram_tensor("wvt", [128, 512], BF16, kind="ExternalInput").ap()
    w1t_d = nc.dram_tensor("w1t", [128, 512], BF16, kind="ExternalInput").ap()
    w2t_d = nc.dram_tensor("w2t", [128, 512], BF16, kind="ExternalInput").ap()
    bq2_d = nc.dram_tensor("bq2", [1, 128], BF16, kind="ExternalInput").ap()
    bk2_d = nc.dram_tensor("bk2", [1, 128], BF16, kind="ExternalInput").ap()
    bv_d = nc.dram_tensor("bv", [1, 256], BF16, kind="ExternalInput").ap()
    b1_d = nc.dram_tensor("b1", [1, 256], BF16, kind="ExternalInput").ap()
    b2_d = nc.dram_tensor("b2", [1, 256], BF16, kind="ExternalInput").ap()
    ones_d = nc.dram_tensor("ones", [1, 512], BF16, kind="ExternalInput").ap()

    y_d = nc.dram_tensor("y", [C, IS], F32, kind="ExternalOutput").ap()

    with tile.TileContext(nc) as tc:
        with (
            tc.tile_pool(name="const", bufs=1) as cpool,
            tc.tile_pool(name="vt", bufs=1) as vtpool,
            tc.tile_pool(name="qk", bufs=1) as qkpool,
            tc.tile_pool(name="outsb", bufs=1) as outpool,
            tc.tile_pool(name="den", bufs=1) as denpool,
            tc.tile_pool(name="dram", bufs=1, space="DRAM") as dram,
        ):
            def cload(nm, shape, dtype, src):
                t = cpool.tile(shape, dtype, name=nm)
                nc.sync.dma_start(t[:], src)
                return t

            ones_sb = cload("ones_sb", [1, 512], BF16, ones_d[:])
            wq2t_sb = cload("wq2t_sb", [128, 256], BF16, wq2t_d[:])
            wk2t_sb = cload("wk2t_sb", [128, 256], BF16, wk2t_d[:])
            wvt_sb = cload("wvt_sb", [128, 512], BF16, wvt_d[:])
            w1t_sb = cload("w1t_sb", [128, 512], BF16, w1t_d[:])
            w2t_sb = cload("w2t_sb", [128, 512], BF16, w2t_d[:])
            bq2_sb = cload("bq2_sb", [1, 128], BF16, bq2_d[:])
            bk2_sb = cload("bk2_sb", [1, 128], BF16, bk2_d[:])
            bv_sb = cload("bv_sb", [1, 256], BF16, bv_d[:])
            b1_sb = cload("b1_sb", [1, 256], BF16, b1_d[:])
            b2_sb = cload("b2_sb", [1, 256], BF16, b2_d[:])

            q2_sb = qkpool.tile([128, IS], BF16)     # [q;q] stacked, d on partitions
            k2_sb = qkpool.tile([128, N], BF16)      # [k;k] stacked
            vt_sb = vtpool.tile([128, NJT * 256], BF16)   # v^T, j on partitions
            vts_sb = vtpool.tile([128, NJT * 256], BF16)  # v^T / den
            den_sb = denpool.tile([128, NJT], F32)
            dsum_sb = denpool.tile([128, NJT], F32)
            rden_sb = denpool.tile([128, NJT], F32)
            out_sb = [outpool.tile([128, IS], BF16, name=f"out_sb{i}") for i in range(2)]

            # ---- Phase 1: projections q2 = [WQ;WQ]x + b + PE, k2, v^T ----
            with (
                tc.tile_pool(name="x", bufs=1) as xpool,
                tc.tile_pool(name="pe", bufs=1) as pepool,
                tc.tile_pool(name="psA", bufs=2, space="PSUM") as psA,
                tc.tile_pool(name="psV", bufs=3, space="PSUM") as psV,
            ):
                xb_sb = [xpool.tile([128, N], BF16, name=f"xb_sb{i}") for i in range(2)]
                xq_sb = [xpool.tile([128, IS], BF16, name=f"xq_sb{i}") for i in range(2)]
                for kt in range(2):
                    nc.sync.dma_start(xb_sb[kt][:], xb_d[kt * 128:(kt + 1) * 128, :])
                    nc.sync.dma_start(xq_sb[kt][:], xq_d[kt * 128:(kt + 1) * 128, :])
                pe2_sb = pepool.tile([128, N], BF16)
                pe2q_sb = pepool.tile([128, IS], BF16)
                nc.sync.dma_start(pe2_sb[:], pe2_d[:])
                nc.sync.dma_start(pe2q_sb[:], pe2q_d[:])

                for ib in range(NIB):
                    sl = bass.ts(ib, 512)
                    ps = psA.tile([128, 512], F32)
                    nc.tensor.matmul(ps[:], bq2_sb[:1, :], ones_sb[:1, :],
                                     start=True, stop=False)
                    for kt in range(2):
                        nc.tensor.matmul(ps[:], wq2t_sb[:, bass.ts(kt, 128)],
                                         xq_sb[kt][:, sl], start=False, stop=(kt == 1))
                    nc.vector.tensor_add(q2_sb[:, sl], ps[:], pe2q_sb[:, sl])

                for jb in range(N // 512):
                    sl = bass.ts(jb, 512)
                    ps = psA.tile([128, 512], F32)
                    nc.tensor.matmul(ps[:], bk2_sb[:1, :], ones_sb[:1, :],
                                     start=True, stop=False)
                    for kt in range(2):
                        nc.tensor.matmul(ps[:], wk2t_sb[:, bass.ts(kt, 128)],
                                         xb_sb[kt][:, sl], start=False, stop=(kt == 1))
                    nc.vector.tensor_add(k2_sb[:, sl], ps[:], pe2_sb[:, sl])

                for jt in range(NJT):
                    ps = psV.tile([128, 256], F32)
                    nc.tensor.matmul(ps[:], ones_sb[:1, :128], bv_sb[:1, :],
                                     start=True, stop=False)
                    for kt in range(2):
                        nc.tensor.matmul(ps[:], xb_sb[kt][:, bass.ts(jt, 128)],
                                         wvt_sb[:, bass.ts(kt, 256)],
                                         start=False, stop=(kt == 1))
                    nc.vector.tensor_copy(vt_sb[:, bass.ts(jt, 256)], ps[:])

            # ---- Phase 2: S[j,i] = (k^T q)/8, attnT = exp(S), den accum ----
            with tc.tile_pool(name="attn", bufs=1) as apool:
                attn_sb = apool.tile([128, NJT * IS], BF16)   # 16 MiB
                with tc.tile_pool(name="psS", bufs=2, space="PSUM") as psS:
                    for jt in range(NJT):
                        half = jt % 2   # row-packed pairs use array rows 0-63 / 64-127
                        hs = slice(64 * half, 64 * (half + 1))
                        ps = psS.tile([128, IS], F32)
                        for ib in range(NIB):
                            nc.tensor.matmul(ps[:, bass.ts(ib, 512)],
                                             k2_sb[hs, bass.ts(jt, 128)],
                                             q2_sb[hs, bass.ts(ib, 512)],
                                             start=True, stop=True)
                        nc.scalar.activation(attn_sb[:, bass.ts(jt, IS)], ps[:],
                                             AF.Exp, scale=0.125,
                                             accum_out=den_sb[:, jt:jt + 1])

                # den AllReduce within the core pair, then fold 1/den into v^T
                den_in = dram.tile([128, NJT], F32)
                den_out = dram.tile([128, NJT], F32)
                nc.sync.dma_start(den_in[:], den_sb[:])
                nc.gpsimd.collective_compute(
                    "AllReduce", mybir.AluOpType.add,
                    replica_groups=REPLICA_GROUPS,
                    ins=[den_in.opt()], outs=[den_out.opt()],
                )
                nc.sync.dma_start(dsum_sb[:], den_out[:])
                nc.vector.reciprocal(rden_sb[:], dsum_sb[:])
                for jt in range(NJT):
                    nc.vector.tensor_scalar_mul(vts_sb[:, bass.ts(jt, 256)],
                                                vt_sb[:, bass.ts(jt, 256)],
                                                rden_sb[:, jt:jt + 1])

                # ---- Phase 3: out[c,i] = sum_j vts[j,c] * attnT[j,i] ----
                with tc.tile_pool(name="psO", bufs=4, space="PSUM") as psO:
                    for mt in range(2):
                        for ib in range(NIB):
                            ps = psO.tile([128, 512], F32)
                            for jt in range(NJT):
                                nc.tensor.matmul(
                                    ps[:],
                                    vts_sb[:, jt * 256 + mt * 128:
                                           jt * 256 + (mt + 1) * 128],
                                    attn_sb[:, jt * IS + ib * 512:
                                            jt * IS + (ib + 1) * 512],
                                    start=(jt == 0), stop=(jt == NJT - 1))
                            nc.vector.tensor_copy(out_sb[mt][:, bass.ts(ib, 512)],
                                                  ps[:])

            # ---- Phase 4: MLP (1x1 -> Mish -> 1x1) + residual ----
            with (
                tc.tile_pool(name="xf", bufs=1) as xfpool,
                tc.tile_pool(name="h", bufs=1) as hpool,
                tc.tile_pool(name="y", bufs=3) as ypool,
                tc.tile_pool(name="psH", bufs=2, space="PSUM") as psH,
                tc.tile_pool(name="psY", bufs=2, space="PSUM") as psY,
            ):
                xf_sb = [xfpool.tile([128, IS], F32, name=f"xf_sb{i}") for i in range(2)]
                for mt in range(2):
                    nc.sync.dma_start(xf_sb[mt][:], xf_d[mt * 128:(mt + 1) * 128, :])
                h_sb = [hpool.tile([128, IS], BF16, name=f"h_sb{i}") for i in range(2)]

                with tc.tile_pool(name="mtmp", bufs=3) as mpool:
                    for mt in range(2):
                        for ib in range(NIB):
                            sl = bass.ts(ib, 512)
                            ps = psH.tile([128, 512], F32)
                            nc.tensor.matmul(ps[:], b1_sb[:1, bass.ts(mt, 128)],
                                             ones_sb[:1, :], start=True, stop=False)
                            for kt in range(2):
                                nc.tensor.matmul(
                                    ps[:],
                                    w1t_sb[:, kt * 256 + mt * 128:
                                           kt * 256 + (mt + 1) * 128],
                                    out_sb[kt][:, sl], start=False, stop=(kt == 1))
                            if use_mish_lut:
                                nc.scalar.activation(h_sb[mt][:, sl], ps[:], AF.Mish)
                            else:
                                # mish(h) = h*(1 - 2/((1+e)^2+1)), e = exp(h)
                                e_t = mpool.tile([128, 512], F32, name="mish_e")
                                p_t = mpool.tile([128, 512], F32, name="mish_p")
                                r_t = mpool.tile([128, 512], F32, name="mish_r")
                                t_t = mpool.tile([128, 512], F32, name="mish_t")
                                nc.scalar.activation(e_t[:], ps[:], AF.Exp)
                                nc.vector.scalar_tensor_tensor(
                                    p_t[:], e_t[:], 2.0, e_t[:],
                                    op0=mybir.AluOpType.add,
                                    op1=mybir.AluOpType.mult)
                                nc.vector.tensor_scalar_add(p_t[:], p_t[:], 2.0)
                                nc.vector.reciprocal(r_t[:], p_t[:])
                                nc.vector.tensor_scalar(
                                    t_t[:], r_t[:], -2.0, 1.0,
                                    op0=mybir.AluOpType.mult,
                                    op1=mybir.AluOpType.add)
                                nc.vector.tensor_tensor(
                                    h_sb[mt][:, sl], ps[:], t_t[:],
                                    op=mybir.AluOpType.mult)

                for mt in range(2):
                    for ib in range(NIB):
                        sl = bass.ts(ib, 512)
                        ps = psY.tile([128, 512], F32)
                        nc.tensor.matmul(ps[:], b2_sb[:1, bass.ts(mt, 128)],
                                         ones_sb[:1, :], start=True, stop=False)
                        for kt in range(2):
                            nc.tensor.matmul(
                                ps[:],
                                w2t_sb[:, kt * 256 + mt * 128:
                                       kt * 256 + (mt + 1) * 128],
                                h_sb[kt][:, sl], start=False, stop=(kt == 1))
                        y_sb = ypool.tile([128, 512], F32)
                        nc.vector.tensor_add(y_sb[:], ps[:], xf_sb[mt][:, sl])
                        nc.sync.dma_start(
                            y_d[mt * 128:(mt + 1) * 128, bass.ts(ib, 512)], y_sb[:])
    nc.finalize()
    return nc


def _to_lhsT_sb(w):
    """[256, M] fp32 -> SBUF layout [128, 2*M] bf16: col block kt holds rows
    kt*128..kt*128+127 of w."""
    k, m = w.shape
    assert k == 256
    return np.ascontiguousarray(
        w.reshape(2, 128, m).transpose(1, 0, 2).reshape(128, 2 * m).astype(bf16))


def _bf(a):
    return np.ascontiguousarray(np.asarray(a, dtype=np.float32).astype(bf16))


def make_in_maps(x, WQ, bQ, WK, bK, WV, bV, PE, W1, b1, W2, b2, n_cores=N_CORES):
    x = np.asarray(x, dtype=np.float32)
    xf3 = np.ascontiguousarray(x.reshape(B, C, N))
    pef = np.asarray(PE, dtype=np.float32).reshape(QK, N)
    pe2 = _bf(np.vstack([pef, pef]))

    shared = {
        "pe2": pe2,
        "wq2t": _to_lhsT_sb(np.concatenate([WQ.T, WQ.T], axis=1)),
        "wk2t": _to_lhsT_sb(np.concatenate([WK.T, WK.T], axis=1)),
        "wvt": _to_lhsT_sb(np.asarray(WV, np.float32).T),
        "w1t": _to_lhsT_sb(np.asarray(W1, np.float32).T),
        "w2t": _to_lhsT_sb(np.asarray(W2, np.float32).T),
        "bq2": _bf(np.concatenate([bQ, bQ])[None, :]),
        "bk2": _bf(np.concatenate([bK, bK])[None, :]),
        "bv": _bf(np.asarray(bV)[None, :]),
        "b1": _bf(np.asarray(b1)[None, :]),
        "b2": _bf(np.asarray(b2)[None, :]),
        "ones": np.ones((1, 512), dtype=bf16),
    }
    in_maps = []
    for core in range(n_cores):
        s, h = core // 2, core % 2
        isl = slice(h * IS, (h + 1) * IS)
        xb = _bf(xf3[s])
        m = dict(shared)
        m["xb"] = xb
        m["xq"] = np.ascontiguousarray(xb[:, isl])
        m["xf"] = np.ascontiguousarray(xf3[s][:, isl])
        m["pe2q"] = np.ascontiguousarray(pe2[:, isl])
        in_maps.append(m)
    return in_maps


def assemble_output(results, n_cores=N_CORES):
    y = np.empty((B, C, N), dtype=np.float32)
    for s in range(B):
        y[s][:, :IS] = results[2 * s]["y"]
        y[s][:, IS:] = results[2 * s + 1]["y"]
    return y.reshape(B, C, H, W)


_PROG = None


def kernel(**inputs) -> np.ndarray:
    global _PROG
    from concourse.bass_utils import run_bass_kernel_spmd
    if _PROG is None:
        _PROG = build_program(N_CORES)
    in_maps = make_in_maps(**inputs)
    res = run_bass_kernel_spmd(_PROG, in_maps, core_ids=list(range(N_CORES)))
    return assemble_output(res.results)


# revision 13
# speedup vs baseline: 1.3737x; 1.3737x over previous
"""AttentionHead kernel for 8 Trainium2 NeuronCores.

Problem (per sample, B=4): x:[256,64,64] -> q/k/v 1x1-conv projections
(+positional encoding on q,k), S = q^T k / 8, softmax over the QUERY axis,
out = attn @ v, then 1x1-conv MLP with Mish + residual.

Sharding: 2 cores per sample, split over the query axis i (2048 queries each).
Softmax normalizes over i, so the per-key denominator den[j] = sum_i exp(S[i,j])
needs one tiny AllReduce per core pair (done in 2 chunks so the latency hides
behind compute); den then folds into v (v/den), everything else is local, and
the output halves are disjoint.

Layout trick: compute S transposed, S[j,i] = (k^T q)[j,i], keys j on partitions.
exp runs PSUM->SBUF with a per-partition accumulate (the denominator for free),
and exp(S)[j,i] is then directly the correct operand layout for both
out[c,i] = sum_j v[c,j]*attnT[j,i] and the MLP — zero on-device transposes.
All matmul operands bf16 (fp32 PSUM accumulation): ~7e-5 rel err.

Bias handling: q/k biases are folded into the positional-encoding tensors on
the host; the v bias is a broadcast tensor added during the PSUM->SBUF move;
b1 rides the Mish exp's per-partition bias; b2 rides the final residual add.
Mish = x*tanh(softplus(x)) is computed as e=exp(x+b1), sp=ln(e+1), t=tanh(sp),
h=(x+b1)*t — three ScalarE LUT passes (batched per table set) + one DVE op.
"""

import numpy as np
import ml_dtypes

import concourse.bass as bass
import concourse.bacc as bacc
import concourse.mybir as mybir
import concourse.tile as tile

BF16 = mybir.dt.bfloat16
F32 = mybir.dt.float32
AF = mybir.ActivationFunctionType
OP = mybir.AluOpType
bf16 = ml_dtypes.bfloat16

B, C, H, W = 4, 256, 64, 64
N = H * W            # 4096 pixels
QK = 64
IS = N // 2          # 2048 queries per core
NJT = N // 128       # 32 key tiles
NIB = IS // 512      # 4 i-blocks
NCH = 2              # den allreduce chunks
JCH = NJT // NCH     # 16 key tiles per chunk
N_CORES = 8
REPLICA_GROUPS = [[0, 1], [2, 3], [4, 5], [6, 7]]


def build_program(n_cores: int = N_CORES, enable_asserts: bool = False) -> bass.Bass:
    nc = bacc.Bacc(
        "TRN2",
        target_bir_lowering=False,
        debug=False,
        enable_asserts=enable_asserts,
        num_devices=n_cores,
    )

    # Per-core inputs (data differs by core; program is identical).
    xb_d = nc.dram_tensor("xb", [C, N], BF16, kind="ExternalInput").ap()
    xq_d = nc.dram_tensor("xq", [C, IS], BF16, kind="ExternalInput").ap()
    xf_d = nc.dram_tensor("xf", [C, IS], F32, kind="ExternalInput").ap()
    pe1q_d = nc.dram_tensor("pe1q", [QK, IS], BF16, kind="ExternalInput").ap()
    # Shared weights (same on all cores).
    pe1_d = nc.dram_tensor("pe1", [QK, N], BF16, kind="ExternalInput").ap()
    wqt_d = nc.dram_tensor("wqt", [128, 128], BF16, kind="ExternalInput").ap()
    wkt_d = nc.dram_tensor("wkt", [128, 128], BF16, kind="ExternalInput").ap()
    wvt_d = nc.dram_tensor("wvt", [128, 512], BF16, kind="ExternalInput").ap()
    w1t_d = nc.dram_tensor("w1t", [128, 512], BF16, kind="ExternalInput").ap()
    w2t_d = nc.dram_tensor("w2t", [128, 512], BF16, kind="ExternalInput").ap()
    bvb_d = nc.dram_tensor("bvb", [128, 256], BF16, kind="ExternalInput").ap()
    b1c_d = nc.dram_tensor("b1c", [128, 2], F32, kind="ExternalInput").ap()
    b2c_d = nc.dram_tensor("b2c", [128, 2], F32, kind="ExternalInput").ap()

    y_d = nc.dram_tensor("y", [C, IS], F32, kind="ExternalOutput").ap()

    with tile.TileContext(nc) as tc:
        with (
            tc.tile_pool(name="const", bufs=1) as cpool,
            tc.tile_pool(name="vt", bufs=1) as vtpool,
            tc.tile_pool(name="qk", bufs=1) as qkpool,
            tc.tile_pool(name="outsb", bufs=1) as outpool,
            tc.tile_pool(name="den", bufs=1) as denpool,
            tc.tile_pool(name="dram", bufs=1, space="DRAM") as dram,
        ):
            def cload(nm, shape, dtype, src):
                t = cpool.tile(shape, dtype, name=nm)
                nc.sync.dma_start(t[:], src)
                return t

            wqt_sb = cload("wqt_sb", [128, 128], BF16, wqt_d[:])
            wkt_sb = cload("wkt_sb", [128, 128], BF16, wkt_d[:])
            wvt_sb = cload("wvt_sb", [128, 512], BF16, wvt_d[:])
            bvb_sb = cload("bvb_sb", [128, 256], BF16, bvb_d[:])
            w1t_sb = cload("w1t_sb", [128, 512], BF16, w1t_d[:])
            w2t_sb = cload("w2t_sb", [128, 512], BF16, w2t_d[:])
            b1c_sb = cload("b1c_sb", [128, 2], F32, b1c_d[:])
            b2c_sb = cload("b2c_sb", [128, 2], F32, b2c_d[:])

            q_sb = qkpool.tile([QK, IS], BF16)     # q, d on partitions
            k_sb = qkpool.tile([QK, N], BF16)      # k, d on partitions
            vt_sb = vtpool.tile([128, NJT * 256], BF16)   # v^T, j on partitions
            vts_sb = vtpool.tile([128, NJT * 256], BF16)  # v^T / den
            den_sb = denpool.tile([128, NJT], F32)
            dsum_sb = denpool.tile([128, NJT], F32)
            rden_sb = denpool.tile([128, NJT], F32)
            out_sb = [outpool.tile([128, IS], BF16, name=f"out_sb{i}") for i in range(2)]

            # ---- Phase 1: projections q = WQ x + bQ + PE, k, v^T ----
            with (
                tc.tile_pool(name="x", bufs=1) as xpool,
                tc.tile_pool(name="pe", bufs=1) as pepool,
                tc.tile_pool(name="psA", bufs=2, space="PSUM") as psA,
                tc.tile_pool(name="psV", bufs=3, space="PSUM") as psV,
            ):
                xq_sb = [xpool.tile([128, IS], BF16, name=f"xq_sb{i}") for i in range(2)]
                xb_sb = [xpool.tile([128, N], BF16, name=f"xb_sb{i}") for i in range(2)]
                pe1q_sb = pepool.tile([QK, IS], BF16)
                pe1_sb = pepool.tile([QK, N], BF16)
                for kt in range(2):
                    nc.sync.dma_start(xq_sb[kt][:], xq_d[kt * 128:(kt + 1) * 128, :])
                nc.sync.dma_start(pe1q_sb[:], pe1q_d[:])
                for kt in range(2):
                    for ch in range(2):
                        nc.sync.dma_start(xb_sb[kt][:, bass.ts(ch, N // 2)],
                                          xb_d[kt * 128:(kt + 1) * 128,
                                               bass.ts(ch, N // 2)])
                nc.sync.dma_start(pe1_sb[:], pe1_d[:])

                for ib in range(NIB):
                    sl = bass.ts(ib, 512)
                    ps = psA.tile([QK, 512], F32)
                    for kt in range(2):
                        nc.tensor.matmul(ps[:], wqt_sb[:, bass.ts(kt, QK)],
                                         xq_sb[kt][:, sl],
                                         start=(kt == 0), stop=(kt == 1))
                    nc.vector.tensor_add(q_sb[:, sl], ps[:], pe1q_sb[:, sl])

                for jb in range(N // 512):
                    sl = bass.ts(jb, 512)
                    ps = psA.tile([QK, 512], F32)
                    for kt in range(2):
                        nc.tensor.matmul(ps[:], wkt_sb[:, bass.ts(kt, QK)],
                                         xb_sb[kt][:, sl],
                                         start=(kt == 0), stop=(kt == 1))
                    nc.vector.tensor_add(k_sb[:, sl], ps[:], pe1_sb[:, sl])

                for jt in range(NJT):
                    ps = psV.tile([128, 256], F32)
                    for kt in range(2):
                        nc.tensor.matmul(ps[:], xb_sb[kt][:, bass.ts(jt, 128)],
                                         wvt_sb[:, bass.ts(kt, 256)],
                                         start=(kt == 0), stop=(kt == 1))
                    nc.vector.tensor_add(vt_sb[:, bass.ts(jt, 256)], ps[:], bvb_sb[:])

            # ---- Phase 2: S[j,i] = (k^T q)/8, attnT = exp(S), den accum ----
            with tc.tile_pool(name="attn", bufs=1) as apool:
                attn_sb = apool.tile([128, NJT * IS], BF16)   # 16 MiB
                with tc.tile_pool(name="psS", bufs=2, space="PSUM") as psS:
                    for jt in range(NJT):
                        ps = psS.tile([128, IS], F32)
                        for ib in range(NIB):
                            nc.tensor.matmul(ps[:, bass.ts(ib, 512)],
                                             k_sb[:, bass.ts(jt, 128)],
                                             q_sb[:, bass.ts(ib, 512)],
                                             start=True, stop=True)
                        nc.scalar.activation(attn_sb[:, bass.ts(jt, IS)], ps[:],
                                             AF.Exp, scale=0.125,
                                             accum_out=den_sb[:, jt:jt + 1])

                # den AllReduce within the core pair (2 chunks, latency hidden),
                # then fold 1/den into v^T
                for ch in range(NCH):
                    csl = bass.ts(ch, JCH)
                    den_in = dram.tile([128, JCH], F32, name=f"den_in{ch}")
                    den_out = dram.tile([128, JCH], F32, name=f"den_out{ch}")
                    nc.sync.dma_start(den_in[:], den_sb[:, csl])
                    nc.gpsimd.collective_compute(
                        "AllReduce", OP.add,
                        replica_groups=REPLICA_GROUPS,
                        ins=[den_in.opt()], outs=[den_out.opt()],
                    )
                    nc.sync.dma_start(dsum_sb[:, csl], den_out[:])
                    nc.vector.reciprocal(rden_sb[:, csl], dsum_sb[:, csl])
                    for jt in range(ch * JCH, (ch + 1) * JCH):
                        nc.vector.tensor_scalar_mul(vts_sb[:, bass.ts(jt, 256)],
                                                    vt_sb[:, bass.ts(jt, 256)],
                                                    rden_sb[:, jt:jt + 1])

                # ---- Phase 3: out[c,i] = sum_j vts[j,c] * attnT[j,i] ----
                # Two j-half visits so the second den chunk's allreduce hides
                # behind the first half's matmuls.
                with tc.tile_pool(name="psO", bufs=1, space="PSUM") as psO:
                    pso = {}
                    for mt in range(2):
                        for ib in range(NIB):
                            pso[mt, ib] = psO.tile([128, 512], F32,
                                                   name=f"pso{mt}{ib}")
                    for half in range(NCH):
                        for mt in range(2):
                            for ib in range(NIB):
                                for jt in range(half * JCH, (half + 1) * JCH):
                                    nc.tensor.matmul(
                                        pso[mt, ib][:],
                                        vts_sb[:, jt * 256 + mt * 128:
                                               jt * 256 + (mt + 1) * 128],
                                        attn_sb[:, jt * IS + ib * 512:
                                                jt * IS + (ib + 1) * 512],
                                        start=(jt == 0), stop=(jt == NJT - 1),
                                        skip_group_check=True)
                    for mt in range(2):
                        for ib in range(NIB):
                            nc.vector.tensor_copy(out_sb[mt][:, bass.ts(ib, 512)],
                                                  pso[mt, ib][:])

            # ---- Phase 4: MLP (1x1 -> Mish -> 1x1) + residual ----
            with (
                tc.tile_pool(name="xf", bufs=1) as xfpool,
                tc.tile_pool(name="h", bufs=1) as hpool,
                tc.tile_pool(name="mtmp", bufs=1) as mpool,
                tc.tile_pool(name="y", bufs=3) as ypool,
                tc.tile_pool(name="psH", bufs=3, space="PSUM") as psH,
                tc.tile_pool(name="psY", bufs=2, space="PSUM") as psY,
            ):
                xf_sb = [xfpool.tile([128, IS], F32, name=f"xf_sb{i}") for i in range(2)]
                for mt in range(2):
                    nc.sync.dma_start(xf_sb[mt][:], xf_d[mt * 128:(mt + 1) * 128, :])
                h_sb = [hpool.tile([128, IS], BF16, name=f"h_sb{i}") for i in range(2)]

                # mish(h) = h*tanh(ln(1+exp(h))), h-bias b1 folded in.
                # All exp/ln (one table set), then all tanh (another) to
                # amortize ACT table loads. u = h+b1 copied out of PSUM on DVE
                # so the h PSUM bank recycles immediately.
                u_ts, sp_ts = {}, {}
                for mt in range(2):
                    for ib in range(NIB):
                        ps = psH.tile([128, 512], F32, name="psh")
                        for kt in range(2):
                            nc.tensor.matmul(
                                ps[:],
                                w1t_sb[:, kt * 256 + mt * 128:
                                       kt * 256 + (mt + 1) * 128],
                                out_sb[kt][:, bass.ts(ib, 512)],
                                start=(kt == 0), stop=(kt == 1))
                        e_t = mpool.tile([128, 512], F32, name="mish_e", bufs=3)
                        nc.scalar.activation(e_t[:], ps[:], AF.Exp,
                                             bias=b1c_sb[:, mt:mt + 1])
                        u_t = mpool.tile([128, 512], F32,
                                         name=f"mish_u{mt}{ib}", bufs=1)
                        nc.vector.tensor_scalar_add(u_t[:], ps[:],
                                                    b1c_sb[:, mt:mt + 1])
                        sp_t = mpool.tile([128, 512], F32,
                                          name=f"mish_sp{mt}{ib}", bufs=1)
                        nc.scalar.activation(sp_t[:], e_t[:], AF.Ln, bias=1.0)
                        u_ts[mt, ib] = u_t
                        sp_ts[mt, ib] = sp_t
                for mt in range(2):
                    for ib in range(NIB):
                        th_t = mpool.tile([128, 512], F32, name="mish_th", bufs=3)
                        nc.scalar.activation(th_t[:], sp_ts[mt, ib][:], AF.Tanh)
                        nc.vector.tensor_tensor(
                            h_sb[mt][:, bass.ts(ib, 512)], u_ts[mt, ib][:],
                            th_t[:], op=OP.mult)

                for mt in range(2):
                    for ib in range(NIB):
                        sl = bass.ts(ib, 512)
                        ps = psY.tile([128, 512], F32)
                        for kt in range(2):
                            nc.tensor.matmul(
                                ps[:],
                                w2t_sb[:, kt * 256 + mt * 128:
                                       kt * 256 + (mt + 1) * 128],
                                h_sb[kt][:, sl], start=(kt == 0), stop=(kt == 1))
                        y_sb = ypool.tile([128, 512], F32)
                        nc.vector.scalar_tensor_tensor(
                            y_sb[:], ps[:], b2c_sb[:, mt:mt + 1],
                            xf_sb[mt][:, sl], op0=OP.add, op1=OP.add)
                        nc.sync.dma_start(
                            y_d[mt * 128:(mt + 1) * 128, bass.ts(ib, 512)], y_sb[:])
    nc.finalize()
    return nc


def _to_lhsT_sb(w):
    """[256, M] fp32 -> SBUF layout [128, 2*M] bf16: col block kt holds rows
    kt*128..kt*128+127 of w."""
    k, m = w.shape
    assert k == 256
    return np.ascontiguousarray(
        w.reshape(2, 128, m).transpose(1, 0, 2).reshape(128, 2 * m).astype(bf16))


def _bf(a):
    return np.ascontiguousarray(np.asarray(a, dtype=np.float32).astype(bf16))


def make_in_maps(x, WQ, bQ, WK, bK, WV, bV, PE, W1, b1, W2, b2, n_cores=N_CORES):
    x = np.asarray(x, dtype=np.float32)
    xf3 = np.ascontiguousarray(x.reshape(B, C, N))
    pef = np.asarray(PE, dtype=np.float32).reshape(QK, N)
    pe1 = _bf(pef + np.asarray(bK, np.float32)[:, None])
    pe1q_full = _bf(pef + np.asarray(bQ, np.float32)[:, None])

    shared = {
        "pe1": pe1,
        "wqt": _to_lhsT_sb(np.asarray(WQ, np.float32).T),
        "wkt": _to_lhsT_sb(np.asarray(WK, np.float32).T),
        "wvt": _to_lhsT_sb(np.asarray(WV, np.float32).T),
        "w1t": _to_lhsT_sb(np.asarray(W1, np.float32).T),
        "w2t": _to_lhsT_sb(np.asarray(W2, np.float32).T),
        "bvb": np.ascontiguousarray(
            np.broadcast_to(_bf(np.asarray(bV)[None, :]), (128, 256))),
        "b1c": np.ascontiguousarray(
            np.asarray(b1, np.float32).reshape(2, 128).T),
        "b2c": np.ascontiguousarray(
            np.asarray(b2, np.float32).reshape(2, 128).T),
    }
    in_maps = []
    for core in range(n_cores):
        s, h = core // 2, core % 2
        isl = slice(h * IS, (h + 1) * IS)
        xb = _bf(xf3[s])
        m = dict(shared)
        m["xb"] = xb
        m["xq"] = np.ascontiguousarray(xb[:, isl])
        m["xf"] = np.ascontiguousarray(xf3[s][:, isl])
        m["pe1q"] = np.ascontiguousarray(pe1q_full[:, isl])
        in_maps.append(m)
    return in_maps


def assemble_output(results, n_cores=N_CORES):
    y = np.empty((B, C, N), dtype=np.float32)
    for s in range(B):
        y[s][:, :IS] = results[2 * s]["y"]
        y[s][:, IS:] = results[2 * s + 1]["y"]
    return y.reshape(B, C, H, W)


_PROG = None


def kernel(**inputs) -> np.ndarray:
    global _PROG
    from concourse.bass_utils import run_bass_kernel_spmd
    if _PROG is None:
        _PROG = build_program(N_CORES)
    in_maps = make_in_maps(**inputs)
    res = run_bass_kernel_spmd(_PROG, in_maps, core_ids=list(range(N_CORES)))
    return assemble_output(res.results)
